# revision 1
# baseline (speedup 1.0000x reference)
"""Trainium2 Bass kernel for nn_DistillationStudentModel (per-view adapter MLP).

Math (per sample b with view v = idx[b]):
    xn  = LayerNorm(x; gamma[v], beta[v])
    h   = gelu(xn @ W1[v] + b1[v])          (erf gelu)
    out = x + h @ W2[v] + b2[v]

Strategy: shard the MLP hidden dim H=8192 across the 8 cores (HS=1024 each).
Every core processes ALL tokens with its H-slice of W1/W2 for all 3 views and
emits a partial MLP output; the host sums the 8 partials and adds the
residual x and b2.

Device-side layout is D-major ("transposed activations"): x is passed as
xT [D, T] so the mm1 contraction dim D sits on SBUF partitions, mm1 emits
hT [HS, T] with the mm2 contraction dim HS already on partitions, and mm2
emits poutT [D, T].

The tiny per-token LayerNorm stats (mu, rstd — 0.1% of the FLOPs) are
precomputed on the host and DMA-broadcast across partitions; the device
applies the normalization, runs both matmuls in bf16 (fp32 PSUM
accumulation), and the erf-GELU on the scalar engine. gamma is folded into
W1 and beta into b1 on the host (b1' = b1 + beta @ W1).

Samples are sorted by view on the host so each view's weight slice is loaded
into SBUF once; the token-tile plan (which view, tile length 512 or 256) is
baked into the compiled kernel from the actual indices.
"""

import numpy as np
import ml_dtypes

import concourse.bass as bass
import concourse.tile as tile
from concourse import bacc, mybir
from concourse.bass_utils import run_bass_kernel_spmd

B, P, D, H, V = 32, 256, 2048, 8192, 3
NCORES = 8
HS = H // NCORES          # per-core hidden slice
T = B * P                 # total tokens
KD = D // 128             # mm1 contraction subtiles
KH = HS // 128            # mm2 contraction subtiles
MH = HS // 128            # mm1 output row tiles
MD = D // 128             # mm2 output row tiles
NT = 512                  # tokens per tile (2 samples)
LN_EPS = 1e-5

f32 = mybir.dt.float32
bf16 = mybir.dt.bfloat16

# debugging/profiling hooks (unused by the grading path)
LAST_NC = None
LAST_RESULT = None


def _tile_plan(idx_sorted):
    """[(view, tok_offset, n_tokens)] with n_tokens in {512, 256}, aligned to
    sorted sample groups so every tile is single-view."""
    counts = np.bincount(idx_sorted, minlength=V)
    plan = []
    off = 0
    for v in range(V):
        n = int(counts[v])
        for _ in range(n // 2):
            plan.append((v, off, 2 * P))
            off += 2 * P
        if n % 2:
            plan.append((v, off, P))
            off += P
    assert off == T
    return plan


def _bcast_ap(handle_ap, toff, nt):
    """[128, nt] partition-stride-0 view of a 1-D DRAM tensor slice."""
    sl = handle_ap[toff:toff + nt]
    return bass.AP(tensor=sl.tensor, offset=sl.offset,
                   ap=[[0, 128]] + [list(p) for p in sl.ap])


def build(plan, repeats=1):
    nc = bacc.Bacc("TRN2", debug=False, num_devices=NCORES)
    x = nc.dram_tensor("xT", [D, T], f32, kind="ExternalInput")
    mu = nc.dram_tensor("mu", [T], f32, kind="ExternalInput")
    rstd = nc.dram_tensor("rstd", [T], f32, kind="ExternalInput")
    w1 = nc.dram_tensor("w1", [V, D, HS], bf16, kind="ExternalInput")
    b1 = nc.dram_tensor("b1", [V, HS], f32, kind="ExternalInput")
    w2 = nc.dram_tensor("w2", [V, HS, D], bf16, kind="ExternalInput")
    out = nc.dram_tensor("poutT", [D, T], f32, kind="ExternalOutput")

    x3 = x[:].rearrange("(k p) t -> p k t", p=128)
    w14 = w1[:].rearrange("v (k p) h -> p v k h", p=128)
    w24 = w2[:].rearrange("v (k p) d -> p v k d", p=128)
    b13 = b1[:].rearrange("v (m p) -> p v m", p=128)
    out3 = out[:].rearrange("(m p) t -> p m t", p=128)
    mu1 = mu[:]
    rstd1 = rstd[:]

    views_in_plan = []
    for v, _, _ in plan:
        if v not in views_in_plan:
            views_in_plan.append(v)

    with tile.TileContext(nc) as tc:
        with (
            tc.tile_pool(name="consts", bufs=1) as consts,
            tc.tile_pool(name="w1pool", bufs=18) as w1pool,
            tc.tile_pool(name="w2pool", bufs=10) as w2pool,
            tc.tile_pool(name="xpool", bufs=8) as xpool,
            tc.tile_pool(name="zpool", bufs=2) as zpool,
            tc.tile_pool(name="hpool", bufs=3) as hpool,
            tc.tile_pool(name="bcpool", bufs=2) as bcpool,
            tc.tile_pool(name="tpool", bufs=4) as tpool,
            tc.tile_pool(name="opool", bufs=4) as opool,
            tc.tile_pool(name="pmm", bufs=8, space="PSUM") as pmm,
        ):
            b1t = consts.tile([128, V, MH], f32)
            nc.sync.dma_start(b1t[:], b13)

            for _rep in range(repeats):
              for v in views_in_plan:
                w1k = [w1pool.tile([128, HS], bf16, tag="w1k", name=f"w1k_{_rep}_{v}_{k}")
                       for k in range(KD)]
                w2k = [w2pool.tile([128, D], bf16, tag="w2k", name=f"w2k_{_rep}_{v}_{k}")
                       for k in range(KH)]
                first_tile = True

                for (pv, toff, nt) in plan:
                    if pv != v:
                        continue
                    ts_ = slice(toff, toff + nt)

                    mean_bc = bcpool.tile([128, NT], f32, tag="mean_bc")
                    rstd_bc = bcpool.tile([128, NT], f32, tag="rstd_bc")
                    nc.sync.dma_start(mean_bc[:, :nt], _bcast_ap(mu1, toff, nt))
                    nc.sync.dma_start(rstd_bc[:, :nt], _bcast_ap(rstd1, toff, nt))

                    zt = zpool.tile([128, KD, NT], bf16, tag="zt")
                    for k in range(KD):
                        xt = xpool.tile([128, NT], f32, tag="xt")
                        nc.sync.dma_start(xt[:, :nt], x3[:, k, ts_])
                        tmp = tpool.tile([128, NT], f32, tag="tmp")
                        nc.vector.tensor_sub(tmp[:, :nt], xt[:, :nt],
                                             mean_bc[:, :nt])
                        nc.vector.tensor_mul(zt[:, k, :nt], tmp[:, :nt],
                                             rstd_bc[:, :nt])
                        if first_tile:
                            # interleave this view's W1 loads with the first
                            # tile's x/z stage so mm1 isn't starved at startup
                            nc.sync.dma_start(w1k[k][:], w14[:, v, k, :])
                    if first_tile:
                        # W2 is first needed by mm2, one mm1-phase later
                        for k in range(KH):
                            nc.sync.dma_start(w2k[k][:], w24[:, v, k, :])
                        first_tile = False

                    ht = hpool.tile([128, KH, NT], bf16, tag="ht")
                    for m in range(MH):
                        ph = pmm.tile([128, NT], f32, tag="mm")
                        for k in range(KD):
                            nc.tensor.matmul(ph[:, :nt],
                                             w1k[k][:, bass.ts(m, 128)],
                                             zt[:, k, :nt],
                                             start=(k == 0), stop=(k == KD - 1))
                        nc.scalar.activation(ht[:, m, :nt], ph[:, :nt],
                                             mybir.ActivationFunctionType.Gelu,
                                             bias=b1t[:, v, m:m + 1], scale=1.0)

                    for dsub in range(MD):
                        po = pmm.tile([128, NT], f32, tag="mm")
                        for k in range(KH):
                            nc.tensor.matmul(po[:, :nt],
                                             w2k[k][:, bass.ts(dsub, 128)],
                                             ht[:, k, :nt],
                                             start=(k == 0), stop=(k == KH - 1))
                        ot = opool.tile([128, NT], f32, tag="ot")
                        nc.vector.tensor_copy(ot[:, :nt], po[:, :nt])
                        nc.sync.dma_start(out3[:, dsub, ts_], ot[:, :nt])
    nc.finalize()
    return nc


def kernel(**inputs):
    x = np.asarray(inputs["vision_features"], dtype=np.float32)    # [B, P, D]
    idx = np.asarray(inputs["student_view_indices"]).astype(np.int64)  # [B]
    gamma = np.asarray(inputs["gamma"], dtype=np.float32)          # [V, D]
    beta = np.asarray(inputs["beta"], dtype=np.float32)            # [V, D]
    W1 = np.asarray(inputs["W1"], dtype=np.float32)                # [V, D, H]
    b1 = np.asarray(inputs["b1"], dtype=np.float32)                # [V, H]
    W2 = np.asarray(inputs["W2"], dtype=np.float32)                # [V, H, D]
    b2 = np.asarray(inputs["b2"], dtype=np.float32)                # [V, D]

    order = np.argsort(idx, kind="stable")
    idx_sorted = idx[order]
    plan = _tile_plan(idx_sorted)

    # host-side folds: gamma into W1 rows, beta into b1
    W1f = gamma[:, :, None] * W1                                   # [V, D, H]
    b1f = b1 + np.einsum("vd,vdh->vh", beta, W1)                   # [V, H]

    xs = x[order].reshape(T, D)                                    # sorted tokens
    xT = np.ascontiguousarray(xs.T)                                # [D, T]

    # per-token LayerNorm stats (fp64 accumulate)
    mu_t = xs.mean(axis=1, dtype=np.float64)
    ex2 = np.einsum("td,td->t", xs.astype(np.float64), xs.astype(np.float64)) / D
    var = ex2 - mu_t * mu_t
    rstd_t = (1.0 / np.sqrt(var + LN_EPS)).astype(np.float32)
    mu_t = mu_t.astype(np.float32)

    W1bf = W1f.astype(ml_dtypes.bfloat16)
    W2bf = W2.astype(ml_dtypes.bfloat16)

    in_maps = []
    for c in range(NCORES):
        hsl = slice(c * HS, (c + 1) * HS)
        in_maps.append({
            "xT": xT,
            "mu": mu_t,
            "rstd": rstd_t,
            "w1": np.ascontiguousarray(W1bf[:, :, hsl]),
            "b1": np.ascontiguousarray(b1f[:, hsl]),
            "w2": np.ascontiguousarray(W2bf[:, hsl, :]),
        })

    nc = build(plan)
    res = run_bass_kernel_spmd(nc, in_maps, core_ids=list(range(NCORES)))
    global LAST_NC, LAST_RESULT
    LAST_NC = nc
    LAST_RESULT = res

    pout = res.results[0]["poutT"].astype(np.float32).copy()
    for c in range(1, NCORES):
        pout += res.results[c]["poutT"]

    out_sorted = xs + pout.T                                       # [T, D]
    out_sorted += b2[np.repeat(idx_sorted, P)]
    out = np.empty((B, P, D), dtype=np.float32)
    out[order] = out_sorted.reshape(B, P, D)
    return out



# revision 2
# speedup vs baseline: 1.2021x; 1.2021x over previous
"""Trainium2 Bass kernel for nn_DistillationStudentModel (per-view adapter MLP).

Math (per sample b with view v = idx[b]):
    xn  = LayerNorm(x; gamma[v], beta[v])
    h   = gelu(xn @ W1[v] + b1[v])          (erf gelu)
    out = x + h @ W2[v] + b2[v]

Sharding: MLP hidden dim H=8192 split across 8 cores (HS=1024 each); every
core processes all tokens against its H-slice and emits a partial MLP
output in bf16; the host sums the partials and adds the residual x and b2.

Both matmuls run as fp8(e4m3) DoubleRow passes (2 fp8 weights per PE cell,
half the cycles per output column of bf16).  To stay within the accuracy
budget each operand is kept as an fp8 hi+lo pair and every matmul is
computed as  hi@hi  (main pass, adjacent-k pairs in the DoubleRow slots)
plus  lo@hi + hi@lo  (cross pass, the hi/lo pair occupying the DoubleRow
slots), dropping only the negligible lo@lo term.

Scale folding keeps every PSUM accumulation at a uniform power-of-2 scale:
x is pre-scaled by 16 and W1 by 256 on the host (PSUM1 = 4096 * preact,
removed by the gelu activation's scale=2^-12); h stays at natural scale
with its lo part unscaled (subnormal fp8 — fine, it is a ~2% residual) and
W2 is pre-scaled by 256 (PSUM2 = 256 * mlp, removed by the eviction's
scale=2^-8).  LayerNorm and the gamma/beta folds run on the host; gelu and
the h hi/lo split run on ACT + DVE.

Samples are sorted by view on the host so each view's weights are loaded
once; the token-tile plan is baked into the compiled program.
"""

import numpy as np
import ml_dtypes

import concourse.bass as bass
import concourse.tile as tile
from concourse import bacc, mybir
from concourse.bass_utils import run_bass_kernel_spmd

B, P, D, H, V = 32, 256, 2048, 8192, 3
NCORES = 8
HS = H // NCORES          # per-core hidden slice
T = B * P                 # total tokens
NC1 = D // 128            # mm1 contraction 128-chunks (16)
NC2 = HS // 128           # mm2 contraction 128-chunks (8)
MH = HS // 128            # mm1 output row tiles (8)
MD = D // 128             # mm2 output row tiles (16)
NT = 512                  # tokens per tile (2 samples)
LN_EPS = 1e-5

f32 = mybir.dt.float32
bf16 = mybir.dt.bfloat16
fp8 = mybir.dt.float8e4
DR = mybir.MatmulPerfMode.DoubleRow
E4 = ml_dtypes.float8_e4m3
GELU = mybir.ActivationFunctionType.Gelu

# fraction of contraction chunks receiving the hi/lo cross correction
F1 = NC1   # mm1: all 16 chunks
F2 = NC2   # mm2: all 8 chunks

# debugging/profiling hooks (unused by the grading path)
LAST_NC = None
LAST_RESULT = None


def _tile_plan(idx_sorted):
    """[(view, tok_offset, n_tokens)] with n_tokens in {512, 256}, aligned to
    sorted sample groups so every tile is single-view."""
    counts = np.bincount(idx_sorted, minlength=V)
    plan = []
    off = 0
    for v in range(V):
        n = int(counts[v])
        for _ in range(n // 2):
            plan.append((v, off, 2 * P))
            off += 2 * P
        if n % 2:
            plan.append((v, off, P))
            off += P
    assert off == T
    return plan


def build(plan):
    nc = bacc.Bacc("TRN2", debug=False, num_devices=NCORES)
    xpk = nc.dram_tensor("xpk", [128, NC1, 2, T], fp8, kind="ExternalInput")
    w1 = nc.dram_tensor("w1", [V, 128, NC1, 2, HS], fp8, kind="ExternalInput")
    b1 = nc.dram_tensor("b1", [128, V, MH], f32, kind="ExternalInput")
    w2 = nc.dram_tensor("w2", [V, 128, NC2, 2, D], fp8, kind="ExternalInput")
    out = nc.dram_tensor("poutT", [128, MD, T], bf16, kind="ExternalOutput")

    views_in_plan = []
    for v, _, _ in plan:
        if v not in views_in_plan:
            views_in_plan.append(v)

    with tile.TileContext(nc) as tc:
        with (
            tc.tile_pool(name="consts", bufs=1) as consts,
            tc.tile_pool(name="w1pool", bufs=2) as w1pool,
            tc.tile_pool(name="w2pool", bufs=2) as w2pool,
            tc.tile_pool(name="xpool", bufs=2) as xpool,
            tc.tile_pool(name="hpool", bufs=2) as hpool,
            tc.tile_pool(name="hbfpool", bufs=2) as hbfpool,
            tc.tile_pool(name="opool", bufs=4) as opool,
            tc.tile_pool(name="pmm", bufs=8, space="PSUM") as pmm,
        ):
            b1t = consts.tile([128, V, MH], f32)
            nc.sync.dma_start(b1t[:], b1[:])

            # issue each view's weight loads one view ahead
            wtiles = {}

            def ensure_weights(v):
                if v in wtiles:
                    return
                w1t = w1pool.tile([128, NC1, 2, HS], fp8, tag="w1t", name=f"w1t_{v}")
                w2t = w2pool.tile([128, NC2, 2, D], fp8, tag="w2t", name=f"w2t_{v}")
                nc.sync.dma_start(w1t[:], w1[v])
                nc.sync.dma_start(w2t[:], w2[v])
                wtiles[v] = (w1t, w2t)

            for vi, v in enumerate(views_in_plan):
                ensure_weights(v)
                w1t, w2t = wtiles[v]
                if vi + 1 < len(views_in_plan):
                    ensure_weights(views_in_plan[vi + 1])

                for (pv, toff, nt) in plan:
                    if pv != v:
                        continue
                    ts_ = slice(toff, toff + nt)

                    xt = xpool.tile([128, NC1, 2, NT], fp8, tag="xt")
                    nc.sync.dma_start(xt[:, :, :, :nt], xpk[:, :, :, ts_])

                    ht = hpool.tile([128, NC2, 2, NT], fp8, tag="ht")
                    for m in range(MH):
                        ph = pmm.tile([128, NT], f32, tag="mm")
                        for c in range(NC1 // 2):
                            nc.tensor.matmul(
                                ph[:, :nt],
                                w1t[:, 2 * c:2 * c + 2, 1, bass.ts(m, 128)],
                                xt[:, 2 * c:2 * c + 2, 0, :nt],
                                start=(c == 0), stop=False, perf_mode=DR)
                        for cp in range(F1):
                            nc.tensor.matmul(
                                ph[:, :nt],
                                w1t[:, cp, :, bass.ts(m, 128)],
                                xt[:, cp, :, :nt],
                                start=False, stop=(cp == F1 - 1), perf_mode=DR)
                        nc.scalar.activation(ht[:, m, 0, :nt], ph[:, :nt], GELU,
                                             bias=b1t[:, v, m:m + 1],
                                             scale=2.0 ** -12)
                        hbf = hbfpool.tile([128, NT], bf16, tag="hbf")
                        nc.scalar.activation(hbf[:, :nt], ph[:, :nt], GELU,
                                             bias=b1t[:, v, m:m + 1],
                                             scale=2.0 ** -12)
                        nc.vector.tensor_sub(ht[:, m, 1, :nt], hbf[:, :nt],
                                             ht[:, m, 0, :nt])

                    for dsub in range(MD):
                        po = pmm.tile([128, NT], f32, tag="mm")
                        for c2 in range(NC2 // 2):
                            nc.tensor.matmul(
                                po[:, :nt],
                                w2t[:, 2 * c2:2 * c2 + 2, 1, bass.ts(dsub, 128)],
                                ht[:, 2 * c2:2 * c2 + 2, 0, :nt],
                                start=(c2 == 0), stop=False, perf_mode=DR)
                        for cp in range(F2):
                            nc.tensor.matmul(
                                po[:, :nt],
                                w2t[:, cp, :, bass.ts(dsub, 128)],
                                ht[:, cp, :, :nt],
                                start=False, stop=(cp == F2 - 1), perf_mode=DR)
                        ot = opool.tile([128, NT], bf16, tag="ot")
                        nc.vector.tensor_scalar_mul(ot[:, :nt], po[:, :nt],
                                                    2.0 ** -8)
                        nc.sync.dma_start(out[:, dsub, ts_], ot[:, :nt])
    nc.finalize()
    return nc


def _q8(a):
    return np.clip(a, -240, 240).astype(E4).astype(np.float32)


def _packx(hi, lo):
    """[D, T] hi/lo -> [128, NC, 2, T] fp8; slot0=hi, slot1=lo."""
    nch = hi.shape[0] // 128
    arr = np.stack([hi.reshape(nch, 128, -1), lo.reshape(nch, 128, -1)], axis=2)
    return np.ascontiguousarray(arr.transpose(1, 0, 2, 3)).astype(E4)


def _packw(hi, lo):
    """[K, M] hi/lo -> [128, NC, 2, M] fp8; slot0=lo, slot1=hi."""
    nch = hi.shape[0] // 128
    arr = np.stack([lo.reshape(nch, 128, -1), hi.reshape(nch, 128, -1)], axis=2)
    return np.ascontiguousarray(arr.transpose(1, 0, 2, 3)).astype(E4)


def kernel(**inputs):
    x = np.asarray(inputs["vision_features"], dtype=np.float32)    # [B, P, D]
    idx = np.asarray(inputs["student_view_indices"]).astype(np.int64)  # [B]
    gamma = np.asarray(inputs["gamma"], dtype=np.float32)          # [V, D]
    beta = np.asarray(inputs["beta"], dtype=np.float32)            # [V, D]
    W1 = np.asarray(inputs["W1"], dtype=np.float32)                # [V, D, H]
    b1 = np.asarray(inputs["b1"], dtype=np.float32)                # [V, H]
    W2 = np.asarray(inputs["W2"], dtype=np.float32)                # [V, H, D]
    b2 = np.asarray(inputs["b2"], dtype=np.float32)                # [V, D]

    order = np.argsort(idx, kind="stable")
    idx_sorted = idx[order]
    plan = _tile_plan(idx_sorted)

    # host-side folds: gamma into W1 rows, beta into b1
    W1f = gamma[:, :, None] * W1                                   # [V, D, H]
    b1f = b1 + np.einsum("vd,vdh->vh", beta, W1)                   # [V, H]

    xs = x[order].reshape(T, D)                                    # sorted tokens
    mu_t = xs.mean(axis=1, dtype=np.float64)
    ex2 = np.einsum("td,td->t", xs.astype(np.float64), xs.astype(np.float64)) / D
    var = ex2 - mu_t * mu_t
    rstd_t = (1.0 / np.sqrt(var + LN_EPS))
    xn = ((xs - mu_t[:, None].astype(np.float32))
          * rstd_t[:, None].astype(np.float32))                    # [T, D]

    # hi/lo fp8 split of 16*xn^T
    xsc = np.ascontiguousarray(16.0 * xn.T)                        # [D, T]
    xh = _q8(xsc)
    xl = _q8(xsc - xh)
    xpk = _packx(xh, xl)

    # per-view bias pack [128, V, MH] (core-sliced later)
    # weights: per-core H-slice, scaled by 256, hi/lo split
    in_maps = []
    for c in range(NCORES):
        hsl = slice(c * HS, (c + 1) * HS)
        w1pk = np.empty((V, 128, NC1, 2, HS), dtype=E4)
        w2pk = np.empty((V, 128, NC2, 2, D), dtype=E4)
        b1pk = np.empty((128, V, MH), dtype=np.float32)
        for v in range(V):
            w1s = 256.0 * W1f[v, :, hsl]                           # [D, HS]
            w1h = _q8(w1s)
            w1l = _q8(w1s - w1h)
            w1pk[v] = _packw(w1h, w1l)
            w2s = 256.0 * W2[v, hsl, :]                            # [HS, D]
            w2h = _q8(w2s)
            w2l = _q8(w2s - w2h)
            w2pk[v] = _packw(w2h, w2l)
            b1pk[:, v, :] = b1f[v, hsl].reshape(MH, 128).T
        in_maps.append({"xpk": xpk, "w1": w1pk, "b1": b1pk, "w2": w2pk})

    nc = build(plan)
    res = run_bass_kernel_spmd(nc, in_maps, core_ids=list(range(NCORES)))
    global LAST_NC, LAST_RESULT
    LAST_NC = nc
    LAST_RESULT = res

    pout = res.results[0]["poutT"].astype(np.float32)
    for c in range(1, NCORES):
        pout += res.results[c]["poutT"].astype(np.float32)
    # [128, MD, T] -> [D, T]
    poutT = pout.transpose(1, 0, 2).reshape(D, T)

    out_sorted = xs + poutT.T                                      # [T, D]
    out_sorted += b2[np.repeat(idx_sorted, P)]
    out = np.empty((B, P, D), dtype=np.float32)
    out[order] = out_sorted.reshape(B, P, D)
    return out


# revision 34
# speedup vs baseline: 1.4911x; 1.2404x over previous
"""Trainium2 Bass kernel for nn_DistillationStudentModel (per-view adapter MLP).

Math (per sample b with view v = idx[b]):
    xn  = LayerNorm(x; gamma[v], beta[v])
    h   = gelu(xn @ W1[v] + b1[v])          (erf gelu)
    out = x + h @ W2[v] + b2[v]

Sharding: MLP hidden dim H=8192 split across 8 cores (HS=1024 each); every
core processes all tokens against its H-slice and emits a partial MLP
output in bf16; the host sums the partials and adds the residual x and b2.

Both matmuls run as fp8(e4m3) DoubleRow passes (2 fp8 weights per PE cell,
half the cycles per output column of bf16).  To stay within the accuracy
budget each operand is kept as an fp8 hi+lo pair and every matmul is
computed as  hi@hi  (main pass, adjacent-k pairs in the DoubleRow slots)
plus  lo@hi + hi@lo  (cross pass, the hi/lo pair occupying the DoubleRow
slots), dropping only the negligible lo@lo term.

Scale folding keeps every PSUM accumulation at a uniform power-of-2 scale:
x is pre-scaled by 16 and W1 by 256 on the host (PSUM1 = 4096 * preact,
removed by the gelu activation's scale=2^-12); h stays at natural scale
with its lo part unscaled (subnormal fp8 — fine, it is a ~2% residual) and
W2 is pre-scaled by 256 (PSUM2 = 256 * mlp, removed by the eviction's
scale=2^-8).  LayerNorm and the gamma/beta folds run on the host; gelu and
the h hi/lo split run on ACT + DVE.

Samples are sorted by view on the host so each view's weights are loaded
once; the token-tile plan is baked into the compiled program.
"""

import numpy as np
import ml_dtypes

import concourse.bass as bass
import concourse.tile as tile
from concourse import bacc, mybir
from concourse.bass_utils import run_bass_kernel_spmd

B, P, D, H, V = 32, 256, 2048, 8192, 3
NCORES = 8
HS = H // NCORES          # per-core hidden slice
T = B * P                 # total tokens
NC1 = D // 128            # mm1 contraction 128-chunks (16)
NC2 = HS // 128           # mm2 contraction 128-chunks (8)
MH = HS // 128            # mm1 output row tiles (8)
MD = D // 128             # mm2 output row tiles (16)
NT = 512                  # tokens per tile (2 samples)
LN_EPS = 1e-5

f32 = mybir.dt.float32
bf16 = mybir.dt.bfloat16
fp8 = mybir.dt.float8e4
DR = mybir.MatmulPerfMode.DoubleRow
E4 = ml_dtypes.float8_e4m3
GELU = mybir.ActivationFunctionType.Gelu

# number of contraction 128-chunks receiving the hi/lo cross correction
# (error budget: rel_err ~ sqrt(7.66e-4*(1-F1/16) + 7.66e-4*(1-F2/8) + 4.5e-6))
F1 = 14    # mm1: 14 of 16 chunks
F2 = 6     # mm2: 6 of 8 chunks

# debugging/profiling hooks (unused by the grading path)
LAST_NC = None
LAST_RESULT = None


def _tile_plan(idx_sorted):
    """[(view, tok_offset, n_tokens)] with n_tokens in {512, 256}, aligned to
    sorted sample groups so every tile is single-view."""
    counts = np.bincount(idx_sorted, minlength=V)
    plan = []
    off = 0
    for v in range(V):
        n = int(counts[v])
        for _ in range(n // 2):
            plan.append((v, off, 2 * P))
            off += 2 * P
        if n % 2:
            plan.append((v, off, P))
            off += P
    assert off == T
    return plan


HSQ = HS // 4             # w1 m-quarter width (2 m-tiles)
DQ = D // 4               # w2 d-quarter width (4 d-tiles)


def build(plan):
    nc = bacc.Bacc("TRN2", debug=False, num_devices=NCORES)
    xpk = nc.dram_tensor("xpk", [128, NC1, 2, T], fp8, kind="ExternalInput")
    # weights are packed m-major in quarters so each quarter is one fat
    # contiguous DMA exactly covering 2 (w1) / 4 (w2) output tiles
    w1 = nc.dram_tensor("w1", [V, 4, 128, NC1, 2, HSQ], fp8,
                        kind="ExternalInput")
    b1 = nc.dram_tensor("b1", [128, V, MH], f32, kind="ExternalInput")
    w2 = nc.dram_tensor("w2", [V, 4, 128, NC2, 2, DQ], fp8,
                        kind="ExternalInput")
    out = nc.dram_tensor("poutT", [128, MD, T], bf16, kind="ExternalOutput")

    views_in_plan = []
    for v, _, _ in plan:
        if v not in views_in_plan:
            views_in_plan.append(v)

    with tile.TileContext(nc) as tc:
        with (
            tc.tile_pool(name="consts", bufs=1) as consts,
            tc.tile_pool(name="w1pool", bufs=8) as w1pool,
            tc.tile_pool(name="w2pool", bufs=8) as w2pool,
            tc.tile_pool(name="xpool", bufs=2) as xpool,
            tc.tile_pool(name="hpool", bufs=2) as hpool,
            tc.tile_pool(name="hbfpool", bufs=2) as hbfpool,
            tc.tile_pool(name="opool", bufs=4) as opool,
            tc.tile_pool(name="pmm", bufs=8, space="PSUM") as pmm,
        ):
            b1t = consts.tile([128, V, MH], f32)
            nc.sync.dma_start(b1t[:], b1[:])

            # PE warm-up: tile 0 is DMA-paced, and every idle gap resets the
            # tensor engine's p-state ramp (full clock only after 3us of
            # continuous execution). Run dummy accumulations into a scratch
            # psum bank (never read) until the first real operands land, so
            # real matmuls start at full clock.
            dummy = consts.tile([128, 2, 640], fp8)
            nc.any.memset(dummy[:], 0)
            pdum = pmm.tile([128, NT], f32, tag="mm", name="warmup_psum")
            NDUM = 34
            for r in range(NDUM):
                nc.tensor.matmul(pdum[:], dummy[:, :, 0:128],
                                 dummy[:, :, 128:640],
                                 start=(r == 0), stop=(r == NDUM - 1),
                                 perf_mode=DR)

            # weight loads ride the Activation engine's HWDGE queue so they
            # never head-of-line block the SP queue's x/out stream; chunked
            # so the consumer can start before the full view has landed
            wtiles = {}

            def alloc_weights(v):
                w1q = [w1pool.tile([128, NC1, 2, HSQ], fp8, tag="w1q",
                                   name=f"w1q_{v}_{q}") for q in range(4)]
                w2q = [w2pool.tile([128, NC2, 2, DQ], fp8, tag="w2q",
                                   name=f"w2q_{v}_{q}") for q in range(4)]
                wtiles[v] = (w1q, w2q)
                return w1q, w2q

            def load_w1(v, eng, qs=range(4), split_first=False):
                w1q = wtiles[v][0]
                for q in qs:
                    if split_first and q == 0:
                        eng.dma_start(w1q[0][:, 0:8, :, :], w1[v, 0][:, 0:8, :, :])
                        eng.dma_start(w1q[0][:, 8:16, :, :], w1[v, 0][:, 8:16, :, :])
                    else:
                        eng.dma_start(w1q[q][:], w1[v, q])

            def load_w2(v, eng):
                w2q = wtiles[v][1]
                for q in range(4):
                    eng.dma_start(w2q[q][:], w2[v, q])

            view_tiles = {v: [(toff, nt) for pv, toff, nt in plan if pv == v]
                          for v in views_in_plan}
            nviews = len(views_in_plan)

            # first view: loads balanced across both DGE queues, ordered by
            # first use (tile 0 is DMA-paced); the DGE queues are
            # out-of-order so parked instructions never block ready ones
            v0 = views_in_plan[0]
            w1q0, w2q0 = alloc_weights(v0)
            load_w1(v0, nc.scalar, qs=[0, 1], split_first=True)
            nc.scalar.dma_start(w2q0[0][:], w2[v0, 0])
            nc.scalar.dma_start(w2q0[1][:], w2[v0, 1])

            # x tiles prefetch one plan-tile ahead on the SP queue
            xtiles = {}
            x_loaded = set()

            def load_x(j, first=False):
                if j >= len(plan) or j in x_loaded:
                    return
                x_loaded.add(j)
                _, toff_, nt_ = plan[j]
                xt_ = xpool.tile([128, NC1, 2, NT], fp8, tag="xt",
                                 name=f"xt_{j}")
                xcuts = [0, 2, 4, 8, 16] if first else [0, 4, 8, 12, 16]
                for a, b in zip(xcuts, xcuts[1:]):
                    nc.sync.dma_start(xt_[:, a:b, :, :nt_],
                                      xpk[:, a:b, :, toff_:toff_ + nt_])
                xtiles[j] = xt_

            # SP queue at startup: x0, w1's back quarters, x1, w2's back
            # quarters (each lands just ahead of its first consumer)
            load_x(0, first=True)
            load_w1(v0, nc.sync, qs=[2, 3])
            load_x(1)
            nc.sync.dma_start(w2q0[2][:], w2[v0, 2])
            nc.sync.dma_start(w2q0[3][:], w2[v0, 3])
            tile_no = 0

            for vi, v in enumerate(views_in_plan):
                w1q, w2q = wtiles[v]

                for ti, (toff, nt) in enumerate(view_tiles[v]):
                    ts_ = slice(toff, toff + nt)

                    xt = xtiles.pop(tile_no)
                    tile_no += 1
                    load_x(tile_no)

                    # prefetch the next view's weights: view 1 early (fresh
                    # pool buffer, never parks); view 2+ at the current
                    # view's last tile, when the buffer it reuses has been
                    # released, so the DMA never parks long on the ACT queue
                    if vi + 1 < nviews:
                        vnext = views_in_plan[vi + 1]
                        issue_at = min(1, len(view_tiles[v]) - 1) if vi == 0 \
                            else len(view_tiles[v]) - 1
                        if ti == issue_at:
                            alloc_weights(vnext)
                            load_w1(vnext, nc.scalar)
                            load_w2(vnext, nc.scalar)

                    ht = hpool.tile([128, NC2, 2, NT], fp8, tag="ht")
                    for m in range(MH):
                        w1m = w1q[m // 2]
                        ms = bass.ts(m % 2, 128)
                        ph = pmm.tile([128, NT], f32, tag="mm")
                        for c in range(NC1 // 2):
                            nc.tensor.matmul(
                                ph[:, :nt],
                                w1m[:, 2 * c:2 * c + 2, 1, ms],
                                xt[:, 2 * c:2 * c + 2, 0, :nt],
                                start=(c == 0), stop=False, perf_mode=DR)
                        for cp in range(F1):
                            nc.tensor.matmul(
                                ph[:, :nt],
                                w1m[:, cp, :, ms],
                                xt[:, cp, :, :nt],
                                start=False, stop=(cp == F1 - 1), perf_mode=DR)
                        nc.scalar.activation(ht[:, m, 0, :nt], ph[:, :nt], GELU,
                                             bias=b1t[:, v, m:m + 1],
                                             scale=2.0 ** -12)
                        if m < F2:
                            # m-tiles >= F2 never feed the mm2 cross pass,
                            # so their lo residual is never read
                            hbf = hbfpool.tile([128, NT], bf16, tag="hbf")
                            nc.scalar.activation(hbf[:, :nt], ph[:, :nt], GELU,
                                                 bias=b1t[:, v, m:m + 1],
                                                 scale=2.0 ** -12)
                            nc.vector.tensor_sub(ht[:, m, 1, :nt], hbf[:, :nt],
                                                 ht[:, m, 0, :nt])

                    for dsub in range(MD):
                        w2d = w2q[dsub // 4]
                        ds = bass.ts(dsub % 4, 128)
                        po = pmm.tile([128, NT], f32, tag="mm")
                        # cross chunks first, then main pairs: the only
                        # instruction touching the last h m-tiles comes last,
                        # giving the ACT/DVE h pipeline maximal slack
                        for cp in range(F2):
                            nc.tensor.matmul(
                                po[:, :nt],
                                w2d[:, cp, :, ds],
                                ht[:, cp, :, :nt],
                                start=(cp == 0), stop=False, perf_mode=DR)
                        for c2 in range(NC2 // 2):
                            nc.tensor.matmul(
                                po[:, :nt],
                                w2d[:, 2 * c2:2 * c2 + 2, 1, ds],
                                ht[:, 2 * c2:2 * c2 + 2, 0, :nt],
                                start=False, stop=(c2 == NC2 // 2 - 1),
                                perf_mode=DR)
                        # evictions batch d-tiles into one out DMA: fewer
                        # HWDGE descriptor-generation serializations on the
                        # SP queue, which otherwise gate psum reuse; smaller
                        # groups on the program's final tile cut the drain
                        gsz = 2 if tile_no == len(plan) else 4
                        if dsub % gsz == 0:
                            ot = opool.tile([128, gsz, NT], bf16,
                                            tag=f"ot{gsz}",
                                            name=f"ot_{tile_no}_{dsub}")
                        nc.vector.tensor_scalar_mul(ot[:, dsub % gsz, :nt],
                                                    po[:, :nt], 2.0 ** -8)
                        if dsub % gsz == gsz - 1:
                            q = dsub // gsz
                            nc.sync.dma_start(
                                out[:, gsz * q:gsz * q + gsz, ts_],
                                ot[:, :, :nt])
    nc.finalize()
    return nc


def _q8(a):
    return np.clip(a, -240, 240).astype(E4).astype(np.float32)


def _packx(hi, lo):
    """[D, T] hi/lo -> [128, NC, 2, T] fp8; slot0=hi, slot1=lo."""
    nch = hi.shape[0] // 128
    arr = np.stack([hi.reshape(nch, 128, -1), lo.reshape(nch, 128, -1)], axis=2)
    return np.ascontiguousarray(arr.transpose(1, 0, 2, 3)).astype(E4)


def _packw(hi, lo):
    """[K, M] hi/lo -> [128, NC, 2, M] fp8; slot0=lo, slot1=hi."""
    nch = hi.shape[0] // 128
    arr = np.stack([lo.reshape(nch, 128, -1), hi.reshape(nch, 128, -1)], axis=2)
    return np.ascontiguousarray(arr.transpose(1, 0, 2, 3)).astype(E4)


def kernel(**inputs):
    x = np.asarray(inputs["vision_features"], dtype=np.float32)    # [B, P, D]
    idx = np.asarray(inputs["student_view_indices"]).astype(np.int64)  # [B]
    gamma = np.asarray(inputs["gamma"], dtype=np.float32)          # [V, D]
    beta = np.asarray(inputs["beta"], dtype=np.float32)            # [V, D]
    W1 = np.asarray(inputs["W1"], dtype=np.float32)                # [V, D, H]
    b1 = np.asarray(inputs["b1"], dtype=np.float32)                # [V, H]
    W2 = np.asarray(inputs["W2"], dtype=np.float32)                # [V, H, D]
    b2 = np.asarray(inputs["b2"], dtype=np.float32)                # [V, D]

    order = np.argsort(idx, kind="stable")
    idx_sorted = idx[order]
    plan = _tile_plan(idx_sorted)

    # host-side folds: gamma into W1 rows, beta into b1
    W1f = gamma[:, :, None] * W1                                   # [V, D, H]
    b1f = b1 + np.einsum("vd,vdh->vh", beta, W1)                   # [V, H]

    xs = x[order].reshape(T, D)                                    # sorted tokens
    mu_t = xs.mean(axis=1, dtype=np.float64)
    ex2 = np.einsum("td,td->t", xs.astype(np.float64), xs.astype(np.float64)) / D
    var = ex2 - mu_t * mu_t
    rstd_t = (1.0 / np.sqrt(var + LN_EPS))
    xn = ((xs - mu_t[:, None].astype(np.float32))
          * rstd_t[:, None].astype(np.float32))                    # [T, D]

    # hi/lo fp8 split of 16*xn^T
    xsc = np.ascontiguousarray(16.0 * xn.T)                        # [D, T]
    xh = _q8(xsc)
    xl = _q8(xsc - xh)
    xpk = _packx(xh, xl)

    # per-view bias pack [128, V, MH] (core-sliced later)
    # weights: per-core H-slice, scaled by 256, hi/lo split
    in_maps = []
    for c in range(NCORES):
        hsl = slice(c * HS, (c + 1) * HS)
        w1pk = np.empty((V, 128, NC1, 2, HS), dtype=E4)
        w2pk = np.empty((V, 128, NC2, 2, D), dtype=E4)
        b1pk = np.empty((128, V, MH), dtype=np.float32)
        for v in range(V):
            w1s = 256.0 * W1f[v, :, hsl]                           # [D, HS]
            w1h = _q8(w1s)
            w1l = _q8(w1s - w1h)
            w1pk[v] = _packw(w1h, w1l)
            w2s = 256.0 * W2[v, hsl, :]                            # [HS, D]
            w2h = _q8(w2s)
            w2l = _q8(w2s - w2h)
            w2pk[v] = _packw(w2h, w2l)
            b1pk[:, v, :] = b1f[v, hsl].reshape(MH, 128).T
        # m-major quartering for fat single-DMA weight quarters
        w1pq = np.ascontiguousarray(
            w1pk.reshape(V, 128, NC1, 2, 4, HSQ).transpose(0, 4, 1, 2, 3, 5))
        w2pq = np.ascontiguousarray(
            w2pk.reshape(V, 128, NC2, 2, 4, DQ).transpose(0, 4, 1, 2, 3, 5))
        in_maps.append({"xpk": xpk, "w1": w1pq, "b1": b1pk, "w2": w2pq})

    nc = build(plan)
    res = run_bass_kernel_spmd(nc, in_maps, core_ids=list(range(NCORES)))
    global LAST_NC, LAST_RESULT
    LAST_NC = nc
    LAST_RESULT = res

    pout = res.results[0]["poutT"].astype(np.float32)
    for c in range(1, NCORES):
        pout += res.results[c]["poutT"].astype(np.float32)
    # [128, MD, T] -> [D, T]
    poutT = pout.transpose(1, 0, 2).reshape(D, T)

    out_sorted = xs + poutT.T                                      # [T, D]
    out_sorted += b2[np.repeat(idx_sorted, P)]
    out = np.empty((B, P, D), dtype=np.float32)
    out[order] = out_sorted.reshape(B, P, D)
    return out


# revision 35
# speedup vs baseline: 1.5261x; 1.0235x over previous
"""Trainium2 Bass kernel for nn_DistillationStudentModel (per-view adapter MLP).

Math (per sample b with view v = idx[b]):
    xn  = LayerNorm(x; gamma[v], beta[v])
    h   = gelu(xn @ W1[v] + b1[v])          (erf gelu)
    out = x + h @ W2[v] + b2[v]

Sharding: MLP hidden dim H=8192 split across 8 cores (HS=1024 each); every
core processes all tokens against its H-slice and emits a partial MLP
output in bf16; the host sums the partials and adds the residual x and b2.

Both matmuls run as fp8(e4m3) DoubleRow passes (2 fp8 weights per PE cell,
half the cycles per output column of bf16).  To stay within the accuracy
budget each operand is kept as an fp8 hi+lo pair and every matmul is
computed as  hi@hi  (main pass, adjacent-k pairs in the DoubleRow slots)
plus  lo@hi + hi@lo  (cross pass, the hi/lo pair occupying the DoubleRow
slots), dropping only the negligible lo@lo term.

Scale folding keeps every PSUM accumulation at a uniform power-of-2 scale:
x is pre-scaled by 16 and W1 by 256 on the host (PSUM1 = 4096 * preact,
removed by the gelu activation's scale=2^-12); h stays at natural scale
with its lo part unscaled (subnormal fp8 — fine, it is a ~2% residual) and
W2 is pre-scaled by 256 (PSUM2 = 256 * mlp, removed by the eviction's
scale=2^-8).  LayerNorm and the gamma/beta folds run on the host; gelu and
the h hi/lo split run on ACT + DVE.

Samples are sorted by view on the host so each view's weights are loaded
once; the token-tile plan is baked into the compiled program.
"""

import numpy as np
import ml_dtypes

import concourse.bass as bass
import concourse.tile as tile
from concourse import bacc, mybir
from concourse.bass_utils import run_bass_kernel_spmd

B, P, D, H, V = 32, 256, 2048, 8192, 3
NCORES = 8
HS = H // NCORES          # per-core hidden slice
T = B * P                 # total tokens
NC1 = D // 128            # mm1 contraction 128-chunks (16)
NC2 = HS // 128           # mm2 contraction 128-chunks (8)
MH = HS // 128            # mm1 output row tiles (8)
MD = D // 128             # mm2 output row tiles (16)
NT = 512                  # tokens per tile (2 samples)
LN_EPS = 1e-5

f32 = mybir.dt.float32
bf16 = mybir.dt.bfloat16
fp8 = mybir.dt.float8e4
DR = mybir.MatmulPerfMode.DoubleRow
E4 = ml_dtypes.float8_e4m3
GELU = mybir.ActivationFunctionType.Gelu

# number of contraction 128-chunks receiving the hi/lo cross correction
# (error budget: rel_err ~ sqrt(7.66e-4*(1-F1/16) + 7.66e-4*(1-F2/8) + 4.5e-6);
# (13,6) measures 1.838e-2 on device, realization-independent to +-2e-6)
F1 = 13    # mm1: 13 of 16 chunks
F2 = 6     # mm2: 6 of 8 chunks

# debugging/profiling hooks (unused by the grading path)
LAST_NC = None
LAST_RESULT = None


def _tile_plan(idx_sorted):
    """[(view, tok_offset, n_tokens)] with n_tokens in {512, 256}, aligned to
    sorted sample groups so every tile is single-view."""
    counts = np.bincount(idx_sorted, minlength=V)
    plan = []
    off = 0
    for v in range(V):
        n = int(counts[v])
        for _ in range(n // 2):
            plan.append((v, off, 2 * P))
            off += 2 * P
        if n % 2:
            plan.append((v, off, P))
            off += P
    assert off == T
    return plan


HSQ = HS // 4             # w1 m-quarter width (2 m-tiles)
DQ = D // 4               # w2 d-quarter width (4 d-tiles)


def build(plan):
    nc = bacc.Bacc("TRN2", debug=False, num_devices=NCORES)
    xpk = nc.dram_tensor("xpk", [128, NC1, 2, T], fp8, kind="ExternalInput")
    # weights are packed m-major in quarters so each quarter is one fat
    # contiguous DMA exactly covering 2 (w1) / 4 (w2) output tiles
    w1 = nc.dram_tensor("w1", [V, 4, 128, NC1, 2, HSQ], fp8,
                        kind="ExternalInput")
    b1 = nc.dram_tensor("b1", [128, V, MH], f32, kind="ExternalInput")
    w2 = nc.dram_tensor("w2", [V, 4, 128, NC2, 2, DQ], fp8,
                        kind="ExternalInput")
    out = nc.dram_tensor("poutT", [128, MD, T], bf16, kind="ExternalOutput")

    views_in_plan = []
    for v, _, _ in plan:
        if v not in views_in_plan:
            views_in_plan.append(v)

    with tile.TileContext(nc) as tc:
        with (
            tc.tile_pool(name="consts", bufs=1) as consts,
            tc.tile_pool(name="w1pool", bufs=8) as w1pool,
            tc.tile_pool(name="w2pool", bufs=8) as w2pool,
            tc.tile_pool(name="xpool", bufs=2) as xpool,
            tc.tile_pool(name="hpool", bufs=2) as hpool,
            tc.tile_pool(name="hbfpool", bufs=2) as hbfpool,
            tc.tile_pool(name="opool", bufs=4) as opool,
            tc.tile_pool(name="pmm", bufs=8, space="PSUM") as pmm,
        ):
            b1t = consts.tile([128, V, MH], f32)
            nc.sync.dma_start(b1t[:], b1[:])

            # PE warm-up: tile 0 is DMA-paced, and every idle gap resets the
            # tensor engine's p-state ramp (full clock only after 3us of
            # continuous execution). Run dummy accumulations into a scratch
            # psum bank (never read) until the first real operands land, so
            # real matmuls start at full clock.
            dummy = consts.tile([128, 2, 640], fp8)
            nc.any.memset(dummy[:], 0)
            pdum = pmm.tile([128, NT], f32, tag="mm", name="warmup_psum")
            NDUM = 34
            for r in range(NDUM):
                nc.tensor.matmul(pdum[:], dummy[:, :, 0:128],
                                 dummy[:, :, 128:640],
                                 start=(r == 0), stop=(r == NDUM - 1),
                                 perf_mode=DR)

            # weight loads ride the Activation engine's HWDGE queue so they
            # never head-of-line block the SP queue's x/out stream; chunked
            # so the consumer can start before the full view has landed
            wtiles = {}

            def alloc_weights(v):
                w1q = [w1pool.tile([128, NC1, 2, HSQ], fp8, tag="w1q",
                                   name=f"w1q_{v}_{q}") for q in range(4)]
                w2q = [w2pool.tile([128, NC2, 2, DQ], fp8, tag="w2q",
                                   name=f"w2q_{v}_{q}") for q in range(4)]
                wtiles[v] = (w1q, w2q)
                return w1q, w2q

            def load_w1(v, eng, qs=range(4), split_first=False):
                w1q = wtiles[v][0]
                for q in qs:
                    if split_first and q == 0:
                        eng.dma_start(w1q[0][:, 0:8, :, :], w1[v, 0][:, 0:8, :, :])
                        eng.dma_start(w1q[0][:, 8:16, :, :], w1[v, 0][:, 8:16, :, :])
                    else:
                        eng.dma_start(w1q[q][:], w1[v, q])

            def load_w2(v, eng):
                w2q = wtiles[v][1]
                for q in range(4):
                    eng.dma_start(w2q[q][:], w2[v, q])

            view_tiles = {v: [(toff, nt) for pv, toff, nt in plan if pv == v]
                          for v in views_in_plan}
            nviews = len(views_in_plan)

            # first view: loads balanced across both DGE queues, ordered by
            # first use (tile 0 is DMA-paced); the DGE queues are
            # out-of-order so parked instructions never block ready ones
            v0 = views_in_plan[0]
            w1q0, w2q0 = alloc_weights(v0)
            load_w1(v0, nc.scalar, qs=[0, 1], split_first=True)
            nc.scalar.dma_start(w2q0[0][:], w2[v0, 0])
            nc.scalar.dma_start(w2q0[1][:], w2[v0, 1])

            # x tiles prefetch one plan-tile ahead on the SP queue
            xtiles = {}
            x_loaded = set()

            def load_x(j, first=False):
                if j >= len(plan) or j in x_loaded:
                    return
                x_loaded.add(j)
                _, toff_, nt_ = plan[j]
                xt_ = xpool.tile([128, NC1, 2, NT], fp8, tag="xt",
                                 name=f"xt_{j}")
                xcuts = [0, 2, 4, 8, 16] if first else [0, 4, 8, 12, 16]
                for a, b in zip(xcuts, xcuts[1:]):
                    nc.sync.dma_start(xt_[:, a:b, :, :nt_],
                                      xpk[:, a:b, :, toff_:toff_ + nt_])
                xtiles[j] = xt_

            # SP queue at startup: x0, w1's back quarters, x1, w2's back
            # quarters (each lands just ahead of its first consumer)
            load_x(0, first=True)
            load_w1(v0, nc.sync, qs=[2, 3])
            load_x(1)
            nc.sync.dma_start(w2q0[2][:], w2[v0, 2])
            nc.sync.dma_start(w2q0[3][:], w2[v0, 3])
            tile_no = 0

            for vi, v in enumerate(views_in_plan):
                w1q, w2q = wtiles[v]

                for ti, (toff, nt) in enumerate(view_tiles[v]):
                    ts_ = slice(toff, toff + nt)

                    xt = xtiles.pop(tile_no)
                    tile_no += 1
                    load_x(tile_no)

                    # prefetch the next view's weights: view 1 early (fresh
                    # pool buffer, never parks); view 2+ at the current
                    # view's last tile, when the buffer it reuses has been
                    # released, so the DMA never parks long on the ACT queue
                    if vi + 1 < nviews:
                        vnext = views_in_plan[vi + 1]
                        issue_at = min(1, len(view_tiles[v]) - 1) if vi == 0 \
                            else len(view_tiles[v]) - 1
                        if ti == issue_at:
                            alloc_weights(vnext)
                            load_w1(vnext, nc.scalar)
                            load_w2(vnext, nc.scalar)

                    ht = hpool.tile([128, NC2, 2, NT], fp8, tag="ht")
                    for m in range(MH):
                        w1m = w1q[m // 2]
                        ms = bass.ts(m % 2, 128)
                        ph = pmm.tile([128, NT], f32, tag="mm")
                        for c in range(NC1 // 2):
                            nc.tensor.matmul(
                                ph[:, :nt],
                                w1m[:, 2 * c:2 * c + 2, 1, ms],
                                xt[:, 2 * c:2 * c + 2, 0, :nt],
                                start=(c == 0), stop=False, perf_mode=DR)
                        for cp in range(F1):
                            nc.tensor.matmul(
                                ph[:, :nt],
                                w1m[:, cp, :, ms],
                                xt[:, cp, :, :nt],
                                start=False, stop=(cp == F1 - 1), perf_mode=DR)
                        nc.scalar.activation(ht[:, m, 0, :nt], ph[:, :nt], GELU,
                                             bias=b1t[:, v, m:m + 1],
                                             scale=2.0 ** -12)
                        if m < F2:
                            # m-tiles >= F2 never feed the mm2 cross pass,
                            # so their lo residual is never read
                            hbf = hbfpool.tile([128, NT], bf16, tag="hbf")
                            nc.scalar.activation(hbf[:, :nt], ph[:, :nt], GELU,
                                                 bias=b1t[:, v, m:m + 1],
                                                 scale=2.0 ** -12)
                            nc.vector.tensor_sub(ht[:, m, 1, :nt], hbf[:, :nt],
                                                 ht[:, m, 0, :nt])

                    for dsub in range(MD):
                        w2d = w2q[dsub // 4]
                        ds = bass.ts(dsub % 4, 128)
                        po = pmm.tile([128, NT], f32, tag="mm")
                        # cross chunks first, then main pairs: the only
                        # instruction touching the last h m-tiles comes last,
                        # giving the ACT/DVE h pipeline maximal slack
                        for cp in range(F2):
                            nc.tensor.matmul(
                                po[:, :nt],
                                w2d[:, cp, :, ds],
                                ht[:, cp, :, :nt],
                                start=(cp == 0), stop=False, perf_mode=DR)
                        for c2 in range(NC2 // 2):
                            nc.tensor.matmul(
                                po[:, :nt],
                                w2d[:, 2 * c2:2 * c2 + 2, 1, ds],
                                ht[:, 2 * c2:2 * c2 + 2, 0, :nt],
                                start=False, stop=(c2 == NC2 // 2 - 1),
                                perf_mode=DR)
                        # evictions batch d-tiles into one out DMA: fewer
                        # HWDGE descriptor-generation serializations on the
                        # SP queue, which otherwise gate psum reuse; smaller
                        # groups on the program's final tile cut the drain
                        gsz = 2 if tile_no == len(plan) else 4
                        if dsub % gsz == 0:
                            ot = opool.tile([128, gsz, NT], bf16,
                                            tag=f"ot{gsz}",
                                            name=f"ot_{tile_no}_{dsub}")
                        nc.vector.tensor_scalar_mul(ot[:, dsub % gsz, :nt],
                                                    po[:, :nt], 2.0 ** -8)
                        if dsub % gsz == gsz - 1:
                            q = dsub // gsz
                            nc.sync.dma_start(
                                out[:, gsz * q:gsz * q + gsz, ts_],
                                ot[:, :, :nt])
    nc.finalize()
    return nc


def _q8(a):
    return np.clip(a, -240, 240).astype(E4).astype(np.float32)


def _packx(hi, lo):
    """[D, T] hi/lo -> [128, NC, 2, T] fp8; slot0=hi, slot1=lo."""
    nch = hi.shape[0] // 128
    arr = np.stack([hi.reshape(nch, 128, -1), lo.reshape(nch, 128, -1)], axis=2)
    return np.ascontiguousarray(arr.transpose(1, 0, 2, 3)).astype(E4)


def _packw(hi, lo):
    """[K, M] hi/lo -> [128, NC, 2, M] fp8; slot0=lo, slot1=hi."""
    nch = hi.shape[0] // 128
    arr = np.stack([lo.reshape(nch, 128, -1), hi.reshape(nch, 128, -1)], axis=2)
    return np.ascontiguousarray(arr.transpose(1, 0, 2, 3)).astype(E4)


def kernel(**inputs):
    x = np.asarray(inputs["vision_features"], dtype=np.float32)    # [B, P, D]
    idx = np.asarray(inputs["student_view_indices"]).astype(np.int64)  # [B]
    gamma = np.asarray(inputs["gamma"], dtype=np.float32)          # [V, D]
    beta = np.asarray(inputs["beta"], dtype=np.float32)            # [V, D]
    W1 = np.asarray(inputs["W1"], dtype=np.float32)                # [V, D, H]
    b1 = np.asarray(inputs["b1"], dtype=np.float32)                # [V, H]
    W2 = np.asarray(inputs["W2"], dtype=np.float32)                # [V, H, D]
    b2 = np.asarray(inputs["b2"], dtype=np.float32)                # [V, D]

    order = np.argsort(idx, kind="stable")
    idx_sorted = idx[order]
    plan = _tile_plan(idx_sorted)

    # host-side folds: gamma into W1 rows, beta into b1
    W1f = gamma[:, :, None] * W1                                   # [V, D, H]
    b1f = b1 + np.einsum("vd,vdh->vh", beta, W1)                   # [V, H]

    xs = x[order].reshape(T, D)                                    # sorted tokens
    mu_t = xs.mean(axis=1, dtype=np.float64)
    ex2 = np.einsum("td,td->t", xs.astype(np.float64), xs.astype(np.float64)) / D
    var = ex2 - mu_t * mu_t
    rstd_t = (1.0 / np.sqrt(var + LN_EPS))
    xn = ((xs - mu_t[:, None].astype(np.float32))
          * rstd_t[:, None].astype(np.float32))                    # [T, D]

    # hi/lo fp8 split of 16*xn^T
    xsc = np.ascontiguousarray(16.0 * xn.T)                        # [D, T]
    xh = _q8(xsc)
    xl = _q8(xsc - xh)
    xpk = _packx(xh, xl)

    # per-view bias pack [128, V, MH] (core-sliced later)
    # weights: per-core H-slice, scaled by 256, hi/lo split
    in_maps = []
    for c in range(NCORES):
        hsl = slice(c * HS, (c + 1) * HS)
        w1pk = np.empty((V, 128, NC1, 2, HS), dtype=E4)
        w2pk = np.empty((V, 128, NC2, 2, D), dtype=E4)
        b1pk = np.empty((128, V, MH), dtype=np.float32)
        for v in range(V):
            w1s = 256.0 * W1f[v, :, hsl]                           # [D, HS]
            w1h = _q8(w1s)
            w1l = _q8(w1s - w1h)
            w1pk[v] = _packw(w1h, w1l)
            w2s = 256.0 * W2[v, hsl, :]                            # [HS, D]
            w2h = _q8(w2s)
            w2l = _q8(w2s - w2h)
            w2pk[v] = _packw(w2h, w2l)
            b1pk[:, v, :] = b1f[v, hsl].reshape(MH, 128).T
        # m-major quartering for fat single-DMA weight quarters
        w1pq = np.ascontiguousarray(
            w1pk.reshape(V, 128, NC1, 2, 4, HSQ).transpose(0, 4, 1, 2, 3, 5))
        w2pq = np.ascontiguousarray(
            w2pk.reshape(V, 128, NC2, 2, 4, DQ).transpose(0, 4, 1, 2, 3, 5))
        in_maps.append({"xpk": xpk, "w1": w1pq, "b1": b1pk, "w2": w2pq})

    nc = build(plan)
    res = run_bass_kernel_spmd(nc, in_maps, core_ids=list(range(NCORES)))
    global LAST_NC, LAST_RESULT
    LAST_NC = nc
    LAST_RESULT = res

    pout = res.results[0]["poutT"].astype(np.float32)
    for c in range(1, NCORES):
        pout += res.results[c]["poutT"].astype(np.float32)
    # [128, MD, T] -> [D, T]
    poutT = pout.transpose(1, 0, 2).reshape(D, T)

    out_sorted = xs + poutT.T                                      # [T, D]
    out_sorted += b2[np.repeat(idx_sorted, P)]
    out = np.empty((B, P, D), dtype=np.float32)
    out[order] = out_sorted.reshape(B, P, D)
    return out
